# revision 30
# baseline (speedup 1.0000x reference)
"""Banded (sliding-window) multi-head attention for Trainium2, 8 NeuronCores.

Reference computation (fp32):
    q = query @ Wq + bq ; k = key @ Wk + bk ; v = value @ Wv + bv   (per-head split)
    scores = q k^T / sqrt(U), masked to |i-j| <= 128, softmax, out = attn @ v

Sharding: 8 cores = 2 batches x 4 sequence chunks of 512 query rows.
Each core gets its query chunk (transposed), a 768-row padded k/v halo chunk
(transposed), all weights, and a precomputed additive band/bounds mask.

Per-core kernel (SPMD, identical program, different data). All matmuls run in
bf16 (full PE rate, overlappable weight loads); accumulation is fp32 in PSUM.
bf16 rounding of q/k/W enters the scores *before* the 1/8 softmax scale, so
its effect on the attention weights is ~0.1%; the output-side bf16 (v, P)
contributes a few e-3 relative error - well under tolerance.

  - q,k projected into [head*unit, seq] layout; v into natural [seq, head*unit]
    with a ones-column per head appended so P@V also yields the softmax denom.
  - scoresT[c, r] = k_h^T q_h per kv-tile c, over only the in-band r-window;
    the additive band/bounds mask is folded in via an extra identity-stationary
    matmul into the same PSUM accumulation group.
  - P = exp(scoresT / 8) on ACT (no max subtraction needed: |scores| <~ 1.5).
  - out[r, u] = P^T @ v_aug on PE; denominators come out in column U.
  - out *= 1/denom on DVE, DMA back per row-tile/head-pair slice.

Emission is interleaved per head-pair (pair m only needs projection m-tile m)
so ACT/DVE attention work overlaps the remaining projections on PE.
"""

import sys

sys.path.insert(0, "/opt/trn_rl_repo")

import numpy as np
from contextlib import ExitStack

import concourse.bass as bass  # noqa: F401
import concourse.tile as tile
from concourse import bacc, mybir
from concourse.bass_utils import run_bass_kernel_spmd

B, S, D = 2, 2048, 512
H, U = 8, 64
LEFT, RIGHT = 128, 128
NCORES = 8
SC = S // (NCORES // B)  # 512 query rows per core
KC = SC + LEFT + RIGHT  # 768 k/v rows per core (halo)
NJ = KC // 128  # 6 kv column tiles
NT = SC // 128  # 4 query row tiles
KD = D // 128  # 4 contraction tiles
MH = D // 128  # 4 head-pair tiles ([hu] dim)
# exact in-band r-window (start, len) per kv tile j
WIN = [(0, 128), (0, 256), (0, 384), (128, 384), (256, 256), (384, 128)]
NEG = -1.0e5

F32 = mybir.dt.float32
BF16 = mybir.dt.bfloat16
AF = mybir.ActivationFunctionType

_DIAG = "full"   # "full" | "dma" (loads only) | "compute" (tiny loads)
_HINTS = False   # branch-prefetch hints on the timing loop
_WARM = False    # pre-loop ACT table load (timing loop only)
_QCOPY_ACT = False  # q-projection psum->sbuf copy on ACT instead of DVE


def _emit(ctx: ExitStack, tc: "tile.TileContext", io, loop_k=None):
    sb = ctx.enter_context(tc.tile_pool(name="sb", bufs=1))
    sbr = ctx.enter_context(tc.tile_pool(name="sbr", bufs=1))
    psum = ctx.enter_context(tc.tile_pool(name="psum", bufs=1, space="PSUM"))
    if loop_k is not None:
        hints = ()
        if _HINTS:
            hints = (
                mybir.EngineType.PE,
                mybir.EngineType.Activation,
                mybir.EngineType.DVE,
                mybir.EngineType.SP,
                mybir.EngineType.Pool,
            )
        if _WARM:
            # host the one-time ACT table load outside the loop so per-iter
            # time reflects a single-shot execution (which pays it once)
            nc = tc.nc
            warm = sb.tile([1, 2], F32, tag="warm", name="warm")
            nc.vector.memset(warm[:], 0.0)
            nc.scalar.activation(warm[:], warm[:], AF.Exp)
        with tc.For_i(0, loop_k, 1, hint_engines=hints):
            _emit_body(tc, io, sb, sbr, psum)
    else:
        _emit_body(tc, io, sb, sbr, psum)


def _emit_body(tc: "tile.TileContext", io, sb, sbr, psum):
    nc = tc.nc
    qT, kT, vT, Wq, Wk, Wv, bcol, cbf, maskpack, out = io

    def load(dram, n, width, tagp, eng):
        tiles = []
        r = dram.rearrange("(n p) s -> n p s", p=128)
        for k in range(n):
            t = sb.tile([128, width], BF16, tag=f"{tagp}{k}", name=f"{tagp}{k}")
            if _DIAG == "compute":
                eng.dma_start(t[0:1, :], r[k][0:1, :])
            else:
                eng.dma_start(t[:], r[k])
            tiles.append(t)
        return tiles

    # issue order matters: the m=0 q/k projections gate the whole pipeline;
    # interleave weight/activation k-tiles so matmul k can start after 2k+2
    # transfers instead of all eight.
    def load2(dramA, tagA, widthA, dramB, tagB, widthB, eng):
        tilesA, tilesB = [], []
        rA = dramA.rearrange("(n p) s -> n p s", p=128)
        rB = dramB.rearrange("(n p) s -> n p s", p=128)
        for k in range(KD):
            tA = sb.tile([128, widthA], BF16, tag=f"{tagA}{k}", name=f"{tagA}{k}")
            eng.dma_start(tA[:], rA[k])
            tilesA.append(tA)
            tB = sb.tile([128, widthB], BF16, tag=f"{tagB}{k}", name=f"{tagB}{k}")
            eng.dma_start(tB[:], rB[k])
            tilesB.append(tB)
        return tilesA, tilesB

    wq = load(Wq, KD, D, "wq", nc.sync)
    qt = load(qT, KD, SC, "qt", nc.sync)
    kt = load(kT, KD, KC, "kt", nc.gpsimd)
    wk = load(Wk, KD, D, "wk", nc.gpsimd)
    bc_sb = sb.tile([128, 8], F32, tag="bcol", name="bc_sb")
    nc.sync.dma_start(bc_sb[:], bcol[:])
    c_sb = sb.tile([1, KC + D], BF16, tag="cbf", name="c_sb")
    nc.sync.dma_start(c_sb[:], cbf[:])
    mp_sb = sb.tile([128, 4 * 384 + 128], BF16, tag="mp", name="mp_sb")
    nc.gpsimd.dma_start(mp_sb[:], maskpack[:])
    ones_sb = c_sb[:, 0:KC]
    bv_sb = c_sb[:, KC : KC + D]
    _mslot = [0, 1, 2, 2, 2, 3]  # j2/j3/j4 share one window pattern
    mask_sb = [mp_sb[:, _mslot[j] * 384 : (_mslot[j] + 1) * 384] for j in range(NJ)]
    id_sb = mp_sb[:, 4 * 384 : 4 * 384 + 128]
    vt = load(vT, KD, KC, "vt", nc.sync)
    wv = load(Wv, KD, D, "wv", nc.gpsimd)

    q_sb, k_sb = [], []

    def proj_qk(m):
        ps = psum.tile([128, SC], F32, tag="ps", bufs=2, name=f"qp{m}")
        for k in range(KD):
            nc.tensor.matmul(
                ps[:], wq[k][:, m * 128 : (m + 1) * 128], qt[k][:],
                start=(k == 0), stop=(k == KD - 1),
            )
        qsb = sb.tile([128, SC], BF16, tag=f"q{m}", name=f"q{m}")
        if _QCOPY_ACT:
            nc.scalar.activation(
                qsb[:], ps[:], AF.Identity, bias=bc_sb[:, m : m + 1]
            )
        else:
            nc.vector.tensor_scalar_add(qsb[:], ps[:], bc_sb[:, m : m + 1])
        q_sb.append(qsb)

        ksb = sb.tile([128, KC], BF16, tag=f"k{m}", name=f"k{m}")
        for c0, cl in ((0, 512), (512, 256)):
            ps = psum.tile([128, cl], F32, tag="ps", bufs=2, name=f"kp{m}_{c0}")
            for k in range(KD):
                nc.tensor.matmul(
                    ps[:], wk[k][:, m * 128 : (m + 1) * 128],
                    kt[k][:, c0 : c0 + cl], start=(k == 0), stop=(k == KD - 1),
                )
            nc.vector.tensor_scalar_add(
                ksb[:, c0 : c0 + cl], ps[:], bc_sb[:, 4 + m : 5 + m]
            )
        k_sb.append(ksb)

    # v in natural [seq, hu] layout, 65 cols/head (65th = 1.0)
    v_sb = []

    def proj_v(m):
        vs = sbr.tile([128, H * (U + 1)], BF16, tag=f"v{m}", name=f"v{m}")
        vs3 = vs.rearrange("p (h u) -> p h u", h=H)
        nc.vector.memset(vs3[:, :, U : U + 1], 1.0)
        ps = psum.tile([128, D], F32, tag="ps", bufs=2, name=f"vp{m}")
        for k in range(KD):
            nc.tensor.matmul(
                ps[:], vt[k][:, m * 128 : (m + 1) * 128], wv[k][:],
                start=(k == 0), stop=False,
            )
        nc.tensor.matmul(
            ps[:], ones_sb[0:1, 0:128], bv_sb[0:1, :], start=False, stop=True
        )
        nc.vector.tensor_copy(vs3[:, :, 0:U], ps.rearrange("p (h u) -> p h u", h=H))
        v_sb.append(vs)

    out_sb = [sb.tile([128, D], BF16, tag=f"o{t}", name=f"o{t}") for t in range(NT)]
    if _DIAG in ("nopv", "scoresonly", "projonly"):
        for t in range(NT):
            nc.gpsimd.memset(out_sb[t][:], 0.0)
    pts = {}

    def scores_exp_pair(pair, j):
        # both heads' score windows in one 2-bank PSUM tile (offsets 0 / 512),
        # one shared-mask ldweights, one exp instruction for the pair
        m = pair[0] // 2
        w0, wl = WIN[j]
        sp = psum.tile([128, 1024], F32, tag="sc2", bufs=3, name=f"s{m}_{j}")
        if _DIAG != "nomask":
            for hh in (0, 1):
                nc.tensor.matmul(
                    sp[:, hh * 512 : hh * 512 + wl], id_sb[:], mask_sb[j][:, 0:wl],
                    start=True, stop=False,
                )
        for hh in (0, 1):
            dh = hh * 64
            nc.tensor.matmul(
                sp[:, hh * 512 : hh * 512 + wl],
                k_sb[m][dh : dh + 64, j * 128 : (j + 1) * 128],
                q_sb[m][dh : dh + 64, w0 : w0 + wl],
                start=(_DIAG == "nomask"), stop=True,
            )
        if _DIAG == "scoresonly":
            return
        pt = sbr.tile([128, 2, 384], BF16, tag="pt", bufs=7, name=f"pt{m}_{j}")
        sp3 = sp.rearrange("p (h c) -> p h c", h=2)
        nc.scalar.activation(
            pt[:, :, 0:wl], sp3[:, :, 0:wl], AF.Exp, scale=1.0 / 8.0
        )
        for hh in (0, 1):
            pts[(pair[hh], j)] = pt[:, hh, :]

    def pv_pair(pair, t):
        if _DIAG in ("nopv", "scoresonly", "projonly"):
            return
        # both heads of the pair share one PSUM bank: [128, 2*65]
        op = psum.tile([128, 2 * (U + 1)], F32, tag="ps", bufs=2,
                       name=f"ov{pair[0]}_{t}")
        for hh, h in enumerate(pair):
            for i, j in enumerate((t, t + 1, t + 2)):
                w0, _ = WIN[j]
                nc.tensor.matmul(
                    op[:, hh * (U + 1) : (hh + 1) * (U + 1)],
                    pts[(h, j)][:, t * 128 - w0 : t * 128 - w0 + 128],
                    v_sb[j][:, h * (U + 1) : (h + 1) * (U + 1)],
                    start=(i == 0), stop=(i == 2),
                )
        op3 = op.rearrange("p (h u) -> p h u", h=2)
        rec = sbr.tile([128, 2], F32, tag="rec", bufs=8, name=f"rec{pair[0]}_{t}")
        nc.vector.reciprocal(rec[:], op3[:, :, U : U + 1])
        m = pair[0] // 2
        ot = out_sb[t][:, m * 128 : (m + 1) * 128].rearrange(
            "p (h u) -> p h u", h=2
        )
        nc.vector.tensor_tensor(
            ot, op3[:, :, 0:U],
            rec[:].rearrange("p (h o) -> p h o", o=1).to_broadcast((128, 2, U)),
            op=mybir.AluOpType.mult,
        )

    def out_dma(t, m):
        eng = nc.sync if (t + m) % 2 == 0 else nc.gpsimd
        eng.dma_start(
            out[t * 128 : (t + 1) * 128, m * 128 : (m + 1) * 128],
            out_sb[t][:, m * 128 : (m + 1) * 128],
        )

    if _DIAG in ("dma", "dma4"):
        zt = sb.tile([128, D], BF16, tag="o0", name="zt")
        nc.vector.memset(zt[:], 0.0)
        for t in range(NT):
            nc.sync.dma_start(out[t * 128 : (t + 1) * 128, :], zt[:])
        return

    # ---- schedule: head-pair m only needs projection m-tile m ----
    proj_qk(0)
    for m in range(NJ):
        proj_v(m)
    for m in range(MH):
        pair = (2 * m, 2 * m + 1)
        for j in range(NJ):
            if _DIAG != "projonly":
                scores_exp_pair(pair, j)
            if j >= 2:
                t = j - 2
                pv_pair(pair, t)
                out_dma(t, m)
        if m + 1 < MH:
            proj_qk(m + 1)
        t = NT - 1
        pv_pair(pair, t)
        out_dma(t, m)


_PROGRAMS = {}


def build_program(loop_k=None):
    key = (loop_k, _DIAG, _HINTS, _WARM, _QCOPY_ACT)
    if key in _PROGRAMS:
        return _PROGRAMS[key]
    nc = bacc.Bacc("TRN2", target_bir_lowering=False, debug=False, num_devices=NCORES)
    io = (
        nc.dram_tensor("qT", [D, SC], BF16, kind="ExternalInput").ap(),
        nc.dram_tensor("kT", [D, KC], BF16, kind="ExternalInput").ap(),
        nc.dram_tensor("vT", [D, KC], BF16, kind="ExternalInput").ap(),
        nc.dram_tensor("Wq", [D, D], BF16, kind="ExternalInput").ap(),
        nc.dram_tensor("Wk", [D, D], BF16, kind="ExternalInput").ap(),
        nc.dram_tensor("Wv", [D, D], BF16, kind="ExternalInput").ap(),
        nc.dram_tensor("bcol", [128, 8], F32, kind="ExternalInput").ap(),
        nc.dram_tensor("cbf", [1, KC + D], BF16, kind="ExternalInput").ap(),
        nc.dram_tensor("maskpack", [128, 4 * 384 + 128], BF16,
                       kind="ExternalInput").ap(),
        nc.dram_tensor("out", [SC, D], BF16, kind="ExternalOutput").ap(),
    )
    with tile.TileContext(nc) as tc:
        with ExitStack() as ctx:
            _emit(ctx, tc, io, loop_k=loop_k)
    nc.compile()
    _PROGRAMS[key] = nc
    return nc


def _core_inputs(query, key, value, Wq, Wk, Wv, bq, bk, bv, b, t):
    import ml_dtypes

    bf = ml_dtypes.bfloat16
    q0 = t * SC
    k0 = q0 - LEFT
    qT = np.ascontiguousarray(query[b, q0 : q0 + SC, :].T).astype(bf)
    kpad = np.zeros((KC, D), np.float32)
    vpad = np.zeros((KC, D), np.float32)
    lo, hi = max(0, k0), min(S, q0 + SC + RIGHT)
    kpad[lo - k0 : hi - k0] = key[b, lo:hi, :]
    vpad[lo - k0 : hi - k0] = value[b, lo:hi, :]
    kT = np.ascontiguousarray(kpad.T).astype(bf)
    vT = np.ascontiguousarray(vpad.T).astype(bf)

    maskpack = np.full((128, 4 * 384 + 128), NEG, np.float32)
    _mslot = [0, 1, 2, 2, 2, 3]
    for j, slot in ((0, 0), (1, 1), (2, 2), (5, 3)):
        w0, wl = WIN[j]
        c_glob = k0 + j * 128 + np.arange(128)
        r_glob = q0 + w0 + np.arange(wl)
        valid = (
            (np.abs(r_glob[None, :] - c_glob[:, None]) <= LEFT)
            & (c_glob[:, None] >= 0)
            & (c_glob[:, None] < S)
        )
        maskpack[:, slot * 384 : slot * 384 + wl] = np.where(valid, 0.0, NEG)
    # verify j3/j4 really match the shared slot-2 pattern
    for j in (3, 4):
        w0, wl = WIN[j]
        c_glob = k0 + j * 128 + np.arange(128)
        r_glob = q0 + w0 + np.arange(wl)
        valid = (
            (np.abs(r_glob[None, :] - c_glob[:, None]) <= LEFT)
            & (c_glob[:, None] >= 0)
            & (c_glob[:, None] < S)
        )
        ref = np.where(valid, 0.0, NEG)
        assert (maskpack[:, 2 * 384 : 2 * 384 + wl] == ref).all(), (t, j)
    maskpack[:, 4 * 384 :] = np.eye(128, dtype=np.float32)

    bcol = np.stack(
        [bq.reshape(4, 128)[m] for m in range(4)]
        + [bk.reshape(4, 128)[m] for m in range(4)], axis=1
    ).astype(np.float32)
    cbf = np.concatenate([np.ones(KC, np.float32), bv.ravel()]).reshape(1, -1)

    return {
        "qT": qT, "kT": kT, "vT": vT,
        "Wq": Wq.astype(bf), "Wk": Wk.astype(bf), "Wv": Wv.astype(bf),
        "bcol": bcol,
        "cbf": cbf.astype(bf),
        "maskpack": maskpack.astype(bf),
    }


def make_in_maps(inputs):
    f = {k: np.asarray(v, dtype=np.float32) for k, v in inputs.items()}
    in_maps = []
    for core in range(NCORES):
        b, t = core // NT, core % NT
        in_maps.append(
            _core_inputs(
                f["query"], f["key"], f["value"],
                f["Wq"], f["Wk"], f["Wv"], f["bq"], f["bk"], f["bv"], b, t,
            )
        )
    return in_maps


def run(inputs, trace=False):
    """Returns (output, BassKernelResults)."""
    nc = build_program()
    in_maps = make_in_maps(inputs)
    res = run_bass_kernel_spmd(nc, in_maps, list(range(NCORES)), trace=trace)
    out = np.empty((B, S, D), np.float32)
    for core in range(NCORES):
        b, t = core // NT, core % NT
        out[b, t * SC : (t + 1) * SC, :] = res.results[core]["out"].astype(
            np.float32
        )
    return out, res


def kernel(**inputs):
    out, _ = run(inputs)
    return out


# revision 34
# speedup vs baseline: 1.0333x; 1.0333x over previous
"""Banded (sliding-window) multi-head attention for Trainium2, 8 NeuronCores.

Reference computation (fp32):
    q = query @ Wq + bq ; k = key @ Wk + bk ; v = value @ Wv + bv   (per-head split)
    scores = q k^T / sqrt(U), masked to |i-j| <= 128, softmax, out = attn @ v

Sharding: 8 cores = 2 batches x 4 sequence chunks of 512 query rows.
Each core gets its query chunk (transposed), a 768-row padded k/v halo chunk
(transposed), all weights, and a precomputed additive band/bounds mask.

Per-core kernel (SPMD, identical program, different data). All matmuls run in
bf16 (full PE rate, overlappable weight loads); accumulation is fp32 in PSUM.
bf16 rounding of q/k/W enters the scores *before* the 1/8 softmax scale, so
its effect on the attention weights is ~0.1%; the output-side bf16 (v, P)
contributes a few e-3 relative error - well under tolerance.

  - q,k projected into [head*unit, seq] layout; v into natural [seq, head*unit]
    with a ones-column per head appended so P@V also yields the softmax denom.
  - scoresT[c, r] = k_h^T q_h per kv-tile c, over only the in-band r-window;
    the additive band/bounds mask is folded in via an extra identity-stationary
    matmul into the same PSUM accumulation group.
  - P = exp(scoresT / 8) on ACT (no max subtraction needed: |scores| <~ 1.5).
  - out[r, u] = P^T @ v_aug on PE; denominators come out in column U.
  - out *= 1/denom on DVE, DMA back per row-tile/head-pair slice.

Emission is interleaved per head-pair (pair m only needs projection m-tile m)
so ACT/DVE attention work overlaps the remaining projections on PE.
"""

import sys

sys.path.insert(0, "/opt/trn_rl_repo")

import numpy as np
from contextlib import ExitStack

import concourse.bass as bass  # noqa: F401
import concourse.tile as tile
from concourse import bacc, mybir
from concourse.bass_utils import run_bass_kernel_spmd

B, S, D = 2, 2048, 512
H, U = 8, 64
LEFT, RIGHT = 128, 128
NCORES = 8
SC = S // (NCORES // B)  # 512 query rows per core
KC = SC + LEFT + RIGHT  # 768 k/v rows per core (halo)
NJ = KC // 128  # 6 kv column tiles
NT = SC // 128  # 4 query row tiles
KD = D // 128  # 4 contraction tiles
MH = D // 128  # 4 head-pair tiles ([hu] dim)
# exact in-band r-window (start, len) per kv tile j
WIN = [(0, 128), (0, 256), (0, 384), (128, 384), (256, 256), (384, 128)]
NEG = -1.0e5

F32 = mybir.dt.float32
BF16 = mybir.dt.bfloat16
F8 = mybir.dt.float8e4
AF = mybir.ActivationFunctionType

_DIAG = "full"   # "full" | "dma" (loads only) | "compute" (tiny loads)
_HINTS = False   # branch-prefetch hints on the timing loop
_WARM = False    # pre-loop ACT table load (timing loop only)
_QCOPY_ACT = False  # q-projection psum->sbuf copy on ACT instead of DVE
_QORDER = False  # mask early on sync, wv mid-gpsimd, vt last on sync
_PSB = True      # sc2 bufs 2 / ps bufs 4 (measured ~1us better than 3/2)
_FP8QK = False   # q/k in fp8e4m3: saves ~4us DMA but rel err 1.6e-2 - too thin


def _emit(ctx: ExitStack, tc: "tile.TileContext", io, loop_k=None):
    sb = ctx.enter_context(tc.tile_pool(name="sb", bufs=1))
    sbr = ctx.enter_context(tc.tile_pool(name="sbr", bufs=1))
    psum = ctx.enter_context(tc.tile_pool(name="psum", bufs=1, space="PSUM"))
    if loop_k is not None:
        hints = ()
        if _HINTS:
            hints = (
                mybir.EngineType.PE,
                mybir.EngineType.Activation,
                mybir.EngineType.DVE,
                mybir.EngineType.SP,
                mybir.EngineType.Pool,
            )
        if _WARM:
            # host the one-time ACT table load outside the loop so per-iter
            # time reflects a single-shot execution (which pays it once)
            nc = tc.nc
            warm = sb.tile([1, 2], F32, tag="warm", name="warm")
            nc.vector.memset(warm[:], 0.0)
            nc.scalar.activation(warm[:], warm[:], AF.Exp)
        with tc.For_i(0, loop_k, 1, hint_engines=hints):
            _emit_body(tc, io, sb, sbr, psum)
    else:
        _emit_body(tc, io, sb, sbr, psum)


def _emit_body(tc: "tile.TileContext", io, sb, sbr, psum):
    nc = tc.nc
    qT, kT, vT, Wq, Wk, Wv, bcol, cbf, maskpack, out = io

    def load(dram, n, width, tagp, eng, dt=BF16):
        tiles = []
        r = dram.rearrange("(n p) s -> n p s", p=128)
        for k in range(n):
            t = sb.tile([128, width], dt, tag=f"{tagp}{k}", name=f"{tagp}{k}")
            if _DIAG == "compute":
                eng.dma_start(t[0:1, :], r[k][0:1, :])
            else:
                eng.dma_start(t[:], r[k])
            tiles.append(t)
        return tiles

    # issue order matters: the m=0 q/k projections gate the whole pipeline;
    # interleave weight/activation k-tiles so matmul k can start after 2k+2
    # transfers instead of all eight.
    def load2(dramA, tagA, widthA, dramB, tagB, widthB, eng):
        tilesA, tilesB = [], []
        rA = dramA.rearrange("(n p) s -> n p s", p=128)
        rB = dramB.rearrange("(n p) s -> n p s", p=128)
        for k in range(KD):
            tA = sb.tile([128, widthA], BF16, tag=f"{tagA}{k}", name=f"{tagA}{k}")
            eng.dma_start(tA[:], rA[k])
            tilesA.append(tA)
            tB = sb.tile([128, widthB], BF16, tag=f"{tagB}{k}", name=f"{tagB}{k}")
            eng.dma_start(tB[:], rB[k])
            tilesB.append(tB)
        return tilesA, tilesB

    qkdt = F8 if _FP8QK else BF16
    wq = load(Wq, KD, D, "wq", nc.sync, qkdt)
    qt = load(qT, KD, SC, "qt", nc.sync, qkdt)
    kt = load(kT, KD, KC, "kt", nc.gpsimd, qkdt)
    wk = load(Wk, KD, D, "wk", nc.gpsimd, qkdt)
    mp_sb = sb.tile([128, 4 * 384 + 128], BF16, tag="mp", name="mp_sb")
    if _QORDER:
        nc.sync.dma_start(mp_sb[:], maskpack[:])
    bc_sb = sb.tile([128, 8], F32, tag="bcol", name="bc_sb")
    nc.sync.dma_start(bc_sb[:], bcol[:])
    c_sb = sb.tile([1, KC + D], BF16, tag="cbf", name="c_sb")
    nc.sync.dma_start(c_sb[:], cbf[:])
    if not _QORDER:
        nc.gpsimd.dma_start(mp_sb[:], maskpack[:])
    ones_sb = c_sb[:, 0:KC]
    bv_sb = c_sb[:, KC : KC + D]
    _mslot = [0, 1, 2, 2, 2, 3]  # j2/j3/j4 share one window pattern
    mask_sb = [mp_sb[:, _mslot[j] * 384 : (_mslot[j] + 1) * 384] for j in range(NJ)]
    id_sb = mp_sb[:, 4 * 384 : 4 * 384 + 128]
    if _QORDER:
        wv = load(Wv, KD, D, "wv", nc.gpsimd)
        vt = load(vT, KD, KC, "vt", nc.sync)
    else:
        vt = load(vT, KD, KC, "vt", nc.sync)
        wv = load(Wv, KD, D, "wv", nc.gpsimd)

    q_sb, k_sb = [], []

    def proj_qk(m):
        ps = psum.tile([128, SC], F32, tag="ps", bufs=(4 if _PSB else 2), name=f"qp{m}")
        for k in range(KD):
            nc.tensor.matmul(
                ps[:], wq[k][:, m * 128 : (m + 1) * 128], qt[k][:],
                start=(k == 0), stop=(k == KD - 1),
            )
        qsb = sb.tile([128, SC], F8 if _FP8QK else BF16, tag=f"q{m}", name=f"q{m}")
        if _QCOPY_ACT:
            nc.scalar.activation(
                qsb[:], ps[:], AF.Identity, bias=bc_sb[:, m : m + 1]
            )
        else:
            nc.vector.tensor_scalar_add(qsb[:], ps[:], bc_sb[:, m : m + 1])
        q_sb.append(qsb)

        ksb = sb.tile([128, KC], F8 if _FP8QK else BF16, tag=f"k{m}", name=f"k{m}")
        for c0, cl in ((0, 512), (512, 256)):
            ps = psum.tile([128, cl], F32, tag="ps", bufs=(4 if _PSB else 2), name=f"kp{m}_{c0}")
            for k in range(KD):
                nc.tensor.matmul(
                    ps[:], wk[k][:, m * 128 : (m + 1) * 128],
                    kt[k][:, c0 : c0 + cl], start=(k == 0), stop=(k == KD - 1),
                )
            nc.vector.tensor_scalar_add(
                ksb[:, c0 : c0 + cl], ps[:], bc_sb[:, 4 + m : 5 + m]
            )
        k_sb.append(ksb)

    # v in natural [seq, hu] layout, 65 cols/head (65th = 1.0)
    v_sb = []

    def proj_v(m):
        vs = sbr.tile([128, H * (U + 1)], BF16, tag=f"v{m}", name=f"v{m}")
        vs3 = vs.rearrange("p (h u) -> p h u", h=H)
        nc.vector.memset(vs3[:, :, U : U + 1], 1.0)
        ps = psum.tile([128, D], F32, tag="ps", bufs=(4 if _PSB else 2), name=f"vp{m}")
        for k in range(KD):
            nc.tensor.matmul(
                ps[:], vt[k][:, m * 128 : (m + 1) * 128], wv[k][:],
                start=(k == 0), stop=False,
            )
        nc.tensor.matmul(
            ps[:], ones_sb[0:1, 0:128], bv_sb[0:1, :], start=False, stop=True
        )
        nc.vector.tensor_copy(vs3[:, :, 0:U], ps.rearrange("p (h u) -> p h u", h=H))
        v_sb.append(vs)

    out_sb = [sb.tile([128, D], BF16, tag=f"o{t}", name=f"o{t}") for t in range(NT)]
    if _DIAG in ("nopv", "scoresonly", "projonly"):
        for t in range(NT):
            nc.gpsimd.memset(out_sb[t][:], 0.0)
    pts = {}

    def scores_exp_pair(pair, j):
        # both heads' score windows in one 2-bank PSUM tile (offsets 0 / 512),
        # one shared-mask ldweights, one exp instruction for the pair
        m = pair[0] // 2
        w0, wl = WIN[j]
        sp = psum.tile([128, 1024], F32, tag="sc2", bufs=(2 if _PSB else 3), name=f"s{m}_{j}")
        if _DIAG != "nomask":
            for hh in (0, 1):
                nc.tensor.matmul(
                    sp[:, hh * 512 : hh * 512 + wl], id_sb[:], mask_sb[j][:, 0:wl],
                    start=True, stop=False,
                )
        for hh in (0, 1):
            dh = hh * 64
            nc.tensor.matmul(
                sp[:, hh * 512 : hh * 512 + wl],
                k_sb[m][dh : dh + 64, j * 128 : (j + 1) * 128],
                q_sb[m][dh : dh + 64, w0 : w0 + wl],
                start=(_DIAG == "nomask"), stop=True,
            )
        if _DIAG == "scoresonly":
            return
        pt = sbr.tile([128, 2, 384], BF16, tag="pt", bufs=7, name=f"pt{m}_{j}")
        sp3 = sp.rearrange("p (h c) -> p h c", h=2)
        nc.scalar.activation(
            pt[:, :, 0:wl], sp3[:, :, 0:wl], AF.Exp,
            scale=(1.0 / 8.0 / 256.0) if _FP8QK else (1.0 / 8.0),
        )
        for hh in (0, 1):
            pts[(pair[hh], j)] = pt[:, hh, :]

    def pv_pair(pair, t):
        if _DIAG in ("nopv", "scoresonly", "projonly"):
            return
        # both heads of the pair share one PSUM bank: [128, 2*65]
        op = psum.tile([128, 2 * (U + 1)], F32, tag="ps", bufs=(4 if _PSB else 2),
                       name=f"ov{pair[0]}_{t}")
        for hh, h in enumerate(pair):
            for i, j in enumerate((t, t + 1, t + 2)):
                w0, _ = WIN[j]
                nc.tensor.matmul(
                    op[:, hh * (U + 1) : (hh + 1) * (U + 1)],
                    pts[(h, j)][:, t * 128 - w0 : t * 128 - w0 + 128],
                    v_sb[j][:, h * (U + 1) : (h + 1) * (U + 1)],
                    start=(i == 0), stop=(i == 2),
                )
        op3 = op.rearrange("p (h u) -> p h u", h=2)
        rec = sbr.tile([128, 2], F32, tag="rec", bufs=8, name=f"rec{pair[0]}_{t}")
        nc.vector.reciprocal(rec[:], op3[:, :, U : U + 1])
        m = pair[0] // 2
        ot = out_sb[t][:, m * 128 : (m + 1) * 128].rearrange(
            "p (h u) -> p h u", h=2
        )
        nc.vector.tensor_tensor(
            ot, op3[:, :, 0:U],
            rec[:].rearrange("p (h o) -> p h o", o=1).to_broadcast((128, 2, U)),
            op=mybir.AluOpType.mult,
        )

    def out_dma(t, m):
        eng = nc.sync if (t + m) % 2 == 0 else nc.gpsimd
        eng.dma_start(
            out[t * 128 : (t + 1) * 128, m * 128 : (m + 1) * 128],
            out_sb[t][:, m * 128 : (m + 1) * 128],
        )

    if _DIAG in ("dma", "dma4"):
        zt = sb.tile([128, D], BF16, tag="o0", name="zt")
        nc.vector.memset(zt[:], 0.0)
        for t in range(NT):
            nc.sync.dma_start(out[t * 128 : (t + 1) * 128, :], zt[:])
        return

    # ---- schedule: head-pair m only needs projection m-tile m ----
    proj_qk(0)
    for m in range(NJ):
        proj_v(m)
    for m in range(MH):
        pair = (2 * m, 2 * m + 1)
        for j in range(NJ):
            if _DIAG != "projonly":
                scores_exp_pair(pair, j)
            if j >= 2:
                t = j - 2
                pv_pair(pair, t)
                out_dma(t, m)
        if m + 1 < MH:
            proj_qk(m + 1)
        t = NT - 1
        pv_pair(pair, t)
        out_dma(t, m)


_PROGRAMS = {}


def build_program(loop_k=None):
    key = (loop_k, _DIAG, _HINTS, _WARM, _QCOPY_ACT, _QORDER, _PSB, _FP8QK)
    if key in _PROGRAMS:
        return _PROGRAMS[key]
    nc = bacc.Bacc("TRN2", target_bir_lowering=False, debug=False, num_devices=NCORES)
    io = (
        nc.dram_tensor("qT", [D, SC], F8 if _FP8QK else BF16, kind="ExternalInput").ap(),
        nc.dram_tensor("kT", [D, KC], F8 if _FP8QK else BF16, kind="ExternalInput").ap(),
        nc.dram_tensor("vT", [D, KC], BF16, kind="ExternalInput").ap(),
        nc.dram_tensor("Wq", [D, D], F8 if _FP8QK else BF16, kind="ExternalInput").ap(),
        nc.dram_tensor("Wk", [D, D], F8 if _FP8QK else BF16, kind="ExternalInput").ap(),
        nc.dram_tensor("Wv", [D, D], BF16, kind="ExternalInput").ap(),
        nc.dram_tensor("bcol", [128, 8], F32, kind="ExternalInput").ap(),
        nc.dram_tensor("cbf", [1, KC + D], BF16, kind="ExternalInput").ap(),
        nc.dram_tensor("maskpack", [128, 4 * 384 + 128], BF16,
                       kind="ExternalInput").ap(),
        nc.dram_tensor("out", [SC, D], BF16, kind="ExternalOutput").ap(),
    )
    with tile.TileContext(nc) as tc:
        with ExitStack() as ctx:
            _emit(ctx, tc, io, loop_k=loop_k)
    nc.compile()
    _PROGRAMS[key] = nc
    return nc


def _core_inputs(query, key, value, Wq, Wk, Wv, bq, bk, bv, b, t):
    import ml_dtypes

    bf = ml_dtypes.bfloat16
    f8 = ml_dtypes.float8_e4m3
    qk = f8 if _FP8QK else bf
    qksc = 4.0 if _FP8QK else 1.0  # sqrt(16): scale x and W each by 4
    q0 = t * SC
    k0 = q0 - LEFT
    qT = np.ascontiguousarray(query[b, q0 : q0 + SC, :].T * qksc).astype(qk)
    kpad = np.zeros((KC, D), np.float32)
    vpad = np.zeros((KC, D), np.float32)
    lo, hi = max(0, k0), min(S, q0 + SC + RIGHT)
    kpad[lo - k0 : hi - k0] = key[b, lo:hi, :]
    vpad[lo - k0 : hi - k0] = value[b, lo:hi, :]
    kT = np.ascontiguousarray(kpad.T * qksc).astype(qk)
    vT = np.ascontiguousarray(vpad.T).astype(bf)

    maskpack = np.full((128, 4 * 384 + 128), NEG, np.float32)
    _mslot = [0, 1, 2, 2, 2, 3]
    for j, slot in ((0, 0), (1, 1), (2, 2), (5, 3)):
        w0, wl = WIN[j]
        c_glob = k0 + j * 128 + np.arange(128)
        r_glob = q0 + w0 + np.arange(wl)
        valid = (
            (np.abs(r_glob[None, :] - c_glob[:, None]) <= LEFT)
            & (c_glob[:, None] >= 0)
            & (c_glob[:, None] < S)
        )
        maskpack[:, slot * 384 : slot * 384 + wl] = np.where(valid, 0.0, NEG)
    # verify j3/j4 really match the shared slot-2 pattern
    for j in (3, 4):
        w0, wl = WIN[j]
        c_glob = k0 + j * 128 + np.arange(128)
        r_glob = q0 + w0 + np.arange(wl)
        valid = (
            (np.abs(r_glob[None, :] - c_glob[:, None]) <= LEFT)
            & (c_glob[:, None] >= 0)
            & (c_glob[:, None] < S)
        )
        ref = np.where(valid, 0.0, NEG)
        assert (maskpack[:, 2 * 384 : 2 * 384 + wl] == ref).all(), (t, j)
    maskpack[:, 4 * 384 :] = np.eye(128, dtype=np.float32)

    bcol = np.stack(
        [bq.reshape(4, 128)[m] * qksc * qksc for m in range(4)]
        + [bk.reshape(4, 128)[m] * qksc * qksc for m in range(4)], axis=1
    ).astype(np.float32)
    cbf = np.concatenate([np.ones(KC, np.float32), bv.ravel()]).reshape(1, -1)

    return {
        "qT": qT, "kT": kT, "vT": vT,
        "Wq": (Wq * qksc).astype(qk), "Wk": (Wk * qksc).astype(qk),
        "Wv": Wv.astype(bf),
        "bcol": bcol,
        "cbf": cbf.astype(bf),
        "maskpack": maskpack.astype(bf),
    }


def make_in_maps(inputs):
    f = {k: np.asarray(v, dtype=np.float32) for k, v in inputs.items()}
    in_maps = []
    for core in range(NCORES):
        b, t = core // NT, core % NT
        in_maps.append(
            _core_inputs(
                f["query"], f["key"], f["value"],
                f["Wq"], f["Wk"], f["Wv"], f["bq"], f["bk"], f["bv"], b, t,
            )
        )
    return in_maps


def run(inputs, trace=False):
    """Returns (output, BassKernelResults)."""
    nc = build_program()
    in_maps = make_in_maps(inputs)
    res = run_bass_kernel_spmd(nc, in_maps, list(range(NCORES)), trace=trace)
    out = np.empty((B, S, D), np.float32)
    for core in range(NCORES):
        b, t = core // NT, core % NT
        out[b, t * SC : (t + 1) * SC, :] = res.results[core]["out"].astype(
            np.float32
        )
    return out, res


def kernel(**inputs):
    out, _ = run(inputs)
    return out
